# revision 6
# baseline (speedup 1.0000x reference)
"""Multi-head attention (B=8, S=1024, D=1024, H=16) on 8 TRN2 NeuronCores.

Sharding: pure data parallel — batch element b on core b. Weights are
broadcast to every core. No collectives.

Per-core pipeline (X: [S, D] for one batch element):
  A. X^T via PE transposes (bf16 matmul vs identity), PSUM->SBUF copies
     split between ACT and DVE.
  C. V = X @ W_v (bf16, natural layout) stored as V_aug[sk, head, 65]
     with a ones column (col 64) that accumulates the softmax denominator
     during PV.
  B0. Q^T/K^T projection for head pair 0 only.
  BD. For each head pair p: attention block for p interleaved (in PE
     program order) with the Q^T/K^T projection of pair p+1. The
     attention block is ACT-bound (8 wide exps per sc chunk ~= 8.6us vs
     5us of PE scores+PV), so pair p+1's projection matmuls fill the PE
     slack instead of forming a separate PE-bound phase.
       scores^T (paired across the 2 heads: 64-row PE groups run
       concurrently), one wide exp per sk on ACT (scale=1/8,
       max-subtraction skipped: scores ~N(0,1)), PV accumulates
       unnormalized out^T (rows 0-63) + denominator (row 64).
     Normalize per (pair, sc, head): DVE copy PSUM->SBUF (fast PSUM bank
     release), reciprocal_approx_fast on the denom row, GpSimd
     partition-broadcast (idle engine), DVE multiply -> attn^T (f32r).
     All bias-adds ride on DVE (tensor_scalar/tensor_tensor), keeping
     ACT exp-only.
  E. Y = attn_out @ W_out + b_out in float32r (full-rate fp32 matmul).
"""

import sys

sys.path.insert(0, "/opt/trn_rl_repo")

import numpy as np

import concourse.bacc as bacc
import concourse.mybir as mybir
from concourse.bass_utils import run_bass_kernel_spmd
from concourse.masks import make_identity
from concourse.tile import TileContext

B = 8
S = 1024
D = 1024
H = 16
DK = D // H  # 64
P = 128
ST = S // P   # 8 s-tiles
DT = D // P   # 8 d-tiles
NTQK = 2 * D // P  # 16 n-tiles for the Q|K part
PAIRS = H // 2     # 8 head pairs
SC = S // 512      # 2 chunks of 512 (matmul free-dim limit)

f32 = mybir.dt.float32
f32r = mybir.dt.float32r
bf16 = mybir.dt.bfloat16
EXP = mybir.ActivationFunctionType.Exp
MULT = mybir.AluOpType.mult
ADD = mybir.AluOpType.add


def build_nc():
    nc = bacc.Bacc()
    X = nc.dram_tensor("X", [S, D], f32, kind="ExternalInput")
    W_in = nc.dram_tensor("W_in", [D, 3 * D], f32, kind="ExternalInput")
    b_in = nc.dram_tensor("b_in", [3 * D], f32, kind="ExternalInput")
    W_out = nc.dram_tensor("W_out", [D, D], f32, kind="ExternalInput")
    b_out = nc.dram_tensor("b_out", [D], f32, kind="ExternalInput")
    out = nc.dram_tensor("out", [S, D], f32, kind="ExternalOutput")

    w_in_kp = W_in.rearrange("(ko p) n -> p ko n", p=P)  # [128, 8, 3072]
    w_out_kp = W_out.rearrange("(ko p) n -> p ko n", p=P)  # [128, 8, 1024]

    with TileContext(nc) as tc:
        const = tc.alloc_tile_pool(name="const", bufs=1)
        # wide PSUM pool: [128, 1024] fp32 = 2 banks/slot, 3 slots; shared
        # by transposes, scores, and the B/C/E projections. pv pool: 2 banks.
        psum = tc.alloc_tile_pool(name="psum", bufs=3, space="PSUM")
        pvps = tc.alloc_tile_pool(name="pvps", bufs=2, space="PSUM")

        identity = const.tile([P, P], bf16)
        make_identity(nc, identity[:])
        bqk = const.tile([P, NTQK], f32)
        nc.sync.dma_start(bqk[:], b_in[0 : 2 * D].rearrange("(o p) -> p o", p=P))
        bv_bc = const.tile([P, D], f32)
        bout_bc = const.tile([P, D], f32)
        ones4 = const.tile([P, ST, H, 1], f32)
        nc.vector.memset(ones4[:], 1.0)

        # ---------------- resident tensors ----------------
        qkT_pool = tc.alloc_tile_pool(name="qkT", bufs=1)
        qkT = qkT_pool.tile([P, NTQK, S], bf16)  # 4 MB
        vaug_pool = tc.alloc_tile_pool(name="vaug", bufs=1)
        v_aug = vaug_pool.tile([P, ST, H, DK + 1], bf16)  # 2.1 MB
        nc.vector.tensor_copy(v_aug[:, :, :, DK : DK + 1], ones4[:])
        xT_pool = tc.alloc_tile_pool(name="xT", bufs=1)
        xT = xT_pool.tile([P, DT, S], bf16)  # 2 MB, lives through BD
        attnT_pool = tc.alloc_tile_pool(name="attnT", bufs=1)
        attnT = attnT_pool.tile([P, DT, S], f32r)  # 4 MB
        wout_pool = tc.alloc_tile_pool(name="wout", bufs=1)
        wout = wout_pool.tile([P, DT, D], f32r)  # 4 MB; DMA during BD

        # B-projection helper: one n-tile (Q or K column block) of W_in.
        def project_qk_tile(nt, wqk_pool):
            w_stage = wqk_pool.tile([P, DT, P], f32, tag="ws")
            nc.sync.dma_start(w_stage[:], w_in_kp[:, :, nt * P : (nt + 1) * P])
            w_tile = wqk_pool.tile([P, DT, P], bf16, tag="w")
            nc.vector.tensor_copy(w_tile[:], w_stage[:])
            ps = psum.tile([P, S], f32, tag="w", name=f"psb{nt}")
            for sc in range(SC):
                for dk in range(DT):
                    nc.tensor.matmul(
                        ps[:, sc * 512 : (sc + 1) * 512],
                        w_tile[:, dk, :],
                        xT[:, dk, sc * 512 : (sc + 1) * 512],
                        start=(dk == 0),
                        stop=(dk == DT - 1),
                    )
            # bias + cast on DVE (keeps ACT exp-only)
            nc.vector.tensor_scalar_add(qkT[:, nt, :], ps[:], bqk[:, nt : nt + 1])

        # ---------------- phase A: X^T (PE transpose, cast to bf16) --------
        with tc.tile_pool(name="xstage", bufs=2) as xstage:
            bv_row = xstage.tile([1, D], f32, tag="brow")
            nc.sync.dma_start(bv_row[:], b_in[None, 2 * D : 3 * D])
            nc.gpsimd.partition_broadcast(bv_bc[:], bv_row[:])
            bout_row = xstage.tile([1, D], f32, tag="brow2")
            nc.sync.dma_start(bout_row[:], b_out[None, :])
            nc.gpsimd.partition_broadcast(bout_bc[:], bout_row[:])
            for si in range(ST):
                x_tile = xstage.tile([P, D], f32, tag="x")
                nc.sync.dma_start(x_tile[:], X[si * P : (si + 1) * P, :])
                xb = xstage.tile([P, D], bf16, tag="xb")
                nc.vector.tensor_copy(xb[:], x_tile[:])
                for dj in range(DT):
                    # transpose as a REGULAR bf16 matmul (x.T @ I): ~4x
                    # faster than fp32 transpose-mode and counts as PE
                    # activity for the HAM clock-gate warmup
                    tp = psum.tile([P, P], f32, tag="w", name="tp")
                    nc.tensor.matmul(
                        tp[:],
                        xb[:, dj * P : (dj + 1) * P],
                        identity[:],
                        start=True,
                        stop=True,
                    )
                    # split PSUM->SBUF copies across ACT and DVE
                    if dj % 2 == 0:
                        nc.scalar.copy(xT[:, dj, si * P : (si + 1) * P], tp[:])
                    else:
                        nc.vector.tensor_copy(
                            xT[:, dj, si * P : (si + 1) * P], tp[:]
                        )

        with (
            tc.tile_pool(name="wv", bufs=1) as wv,
            tc.tile_pool(name="wvstage", bufs=2) as wvstage,
            tc.tile_pool(name="wqk", bufs=2) as wqk_pool,
            tc.tile_pool(name="expp", bufs=3) as expp,
            tc.tile_pool(name="unp", bufs=4) as unp,
            tc.tile_pool(name="rrow", bufs=2) as rrowp,
            tc.tile_pool(name="bcp", bufs=3) as bcp,
        ):
            # V weights streamed per d-tile (f32 stage -> bf16)
            wv_tile = wv.tile([P, DT, D], bf16, tag="wv")
            for dk in range(DT):
                wv_stage = wvstage.tile([P, D], f32, tag="wvs")
                nc.sync.dma_start(wv_stage[:], w_in_kp[:, dk, 2 * D : 3 * D])
                nc.vector.tensor_copy(wv_tile[:, dk, :], wv_stage[:])

            # ---------------- phase C: V projection (bf16, natural) --------
            for st in range(ST):
                ps = psum.tile([P, D], f32, tag="w", name="psc")
                for ncx in range(SC):
                    for dk in range(DT):
                        nc.tensor.matmul(
                            ps[:, ncx * 512 : (ncx + 1) * 512],
                            xT[:, dk, st * P : (st + 1) * P],
                            wv_tile[:, dk, ncx * 512 : (ncx + 1) * 512],
                            start=(dk == 0),
                            stop=(dk == DT - 1),
                        )
                nc.vector.tensor_tensor(
                    v_aug[:, st, :, 0:DK],
                    ps[:].rearrange("p (h d) -> p h d", d=DK),
                    bv_bc[:].rearrange("p (h d) -> p h d", d=DK),
                    ADD,
                )

            # ---------------- B0: Q^T/K^T for pair 0 ----------------
            project_qk_tile(0, wqk_pool)
            project_qk_tile(PAIRS, wqk_pool)

            # W_out prefetch (f32r) — DMA runs during the BD loop
            nc.sync.dma_start(wout[:], w_out_kp[:].bitcast(f32r))

            # ---------------- BD: attention ∥ next-pair projection ---------
            for pr in range(PAIRS):
                for sc in range(SC):
                    pv = [
                        pvps.tile([P, 512], f32, tag="pv", name=f"pv{i}")
                        for i in range(2)
                    ]
                    exps = {}
                    # software pipeline: paired scores(sk) on PE, one wide
                    # exp(sk) on ACT (hh halves share the tile), pv(sk-1)
                    for sk in range(ST + 1):
                        if sk < ST:
                            sps = psum.tile([P, S], f32, tag="w", name="sps")
                            for hh in range(2):
                                base = hh * DK
                                nc.tensor.matmul(
                                    sps[:, hh * 512 : (hh + 1) * 512],
                                    qkT[
                                        base : base + DK,
                                        PAIRS + pr,
                                        sk * P : (sk + 1) * P,
                                    ],
                                    qkT[
                                        base : base + DK,
                                        pr,
                                        sc * 512 : (sc + 1) * 512,
                                    ],
                                    start=True,
                                    stop=True,
                                )
                            ex = expp.tile([P, S], bf16, tag="ex")
                            nc.scalar.activation(
                                ex[:], sps[:], EXP, scale=1.0 / np.sqrt(DK)
                            )
                            exps[sk] = ex
                        if sk >= 1:
                            ex = exps.pop(sk - 1)
                            for hh in range(2):
                                h = 2 * pr + hh
                                nc.tensor.matmul(
                                    pv[hh][0 : DK + 1, :],
                                    v_aug[:, sk - 1, h, :],
                                    ex[:, hh * 512 : (hh + 1) * 512],
                                    start=(sk - 1 == 0),
                                    stop=(sk - 1 == ST - 1),
                                )
                    # normalize: copy out of PSUM (frees the bank), cheap
                    # reciprocal, GpSimd broadcast, DVE multiply
                    for hh in range(2):
                        base = hh * DK
                        un = unp.tile([DK + 1, 512], f32, tag="un")
                        nc.vector.tensor_copy(un[:], pv[hh][0 : DK + 1, :])
                        rrow = rrowp.tile([1, 512], f32, tag="rr")
                        nc.vector.reciprocal_approx_fast(
                            rrow[:], un[DK : DK + 1, :]
                        )
                        # full-tile broadcast (sliced outputs break on HW)
                        bc = bcp.tile([P, 512], f32, tag="bc")
                        nc.gpsimd.partition_broadcast(bc[:], rrow[:])
                        nc.vector.tensor_tensor(
                            attnT[
                                base : base + DK, pr, sc * 512 : (sc + 1) * 512
                            ],
                            un[0:DK, :],
                            bc[0:DK, :],
                            MULT,
                        )
                    # next pair's projection fills the ACT-bound PE slack:
                    # Q-tile after sc0, K-tile after sc1
                    if pr + 1 < PAIRS:
                        project_qk_tile(
                            (pr + 1) if sc == 0 else (PAIRS + pr + 1), wqk_pool
                        )

        # ---------------- phase E: output projection (f32r) ----------------
        with tc.tile_pool(name="ypool", bufs=3) as ypool:
            for st in range(ST):
                ps = psum.tile([P, D], f32, tag="w", name="pse")
                for ncx in range(SC):
                    for dk in range(DT):
                        nc.tensor.matmul(
                            ps[:, ncx * 512 : (ncx + 1) * 512],
                            attnT[:, dk, st * P : (st + 1) * P],
                            wout[:, dk, ncx * 512 : (ncx + 1) * 512],
                            start=(dk == 0),
                            stop=(dk == DT - 1),
                        )
                y = ypool.tile([P, D], f32, tag="y")
                nc.vector.tensor_tensor(y[:], ps[:], bout_bc[:], ADD)
                nc.sync.dma_start(out[st * P : (st + 1) * P, :], y[:])

        for pool in (
            wout_pool,
            attnT_pool,
            xT_pool,
            vaug_pool,
            qkT_pool,
            pvps,
            psum,
            const,
        ):
            pool.release()

    nc.finalize()
    return nc


_NC_CACHE = {}


def get_nc():
    if "nc" not in _NC_CACHE:
        _NC_CACHE["nc"] = build_nc()
    return _NC_CACHE["nc"]


def kernel(X, W_in, b_in, W_out, b_out):
    X = np.ascontiguousarray(np.asarray(X, dtype=np.float32))
    W_in = np.ascontiguousarray(np.asarray(W_in, dtype=np.float32))
    b_in = np.ascontiguousarray(np.asarray(b_in, dtype=np.float32))
    W_out = np.ascontiguousarray(np.asarray(W_out, dtype=np.float32))
    b_out = np.ascontiguousarray(np.asarray(b_out, dtype=np.float32))

    nc = get_nc()
    in_maps = [
        {"X": X[i], "W_in": W_in, "b_in": b_in, "W_out": W_out, "b_out": b_out}
        for i in range(B)
    ]
    res = run_bass_kernel_spmd(nc, in_maps, core_ids=list(range(B)))
    return np.stack([res.results[i]["out"] for i in range(B)], axis=0)
